# revision 13
# baseline (speedup 1.0000x reference)
"""Trainium2 Bass kernel for causal RBF (squared-exponential) attention.

  p_ij = exp(-sm * ||q_i - k_j||^2) causal-masked, out = p @ v (no normalization)
  B,H,S,D = 2,16,2048,64 ; sm = 0.125

Sharding: B*H = 32 heads, 4 heads per core across 8 NeuronCores (head
parallel, no cross-core comm).

Factorization: p = D_q . exp(2*sm*qk - sm*||k||^2) with D_q diagonal in
exp(-sm*||q||^2). D_q is applied to the output rows on the host (O(N));
the device computes, per 128-key x 512-query tile, one fp16 matmul with a
66-deep contraction (rows 0..63 = K^T/Q^T, rows 64/65 = ksq split hi/lo
against -1/2 const rows), then ScalarE evaluates
    pt = exp(0.25 * psum + C)       (psum = qk - ksq/2)
straight PSUM -> SBUF. C is a host-supplied bias chosen from max(qsq) so
that pt <= e^{0.125 qsq + C} stays inside fp16 range (p <= 1 identity).
Diagonal tiles: fully-masked columns are skipped in PV entirely; only the
128x128 triangular corner is masked (VectorE) before the PV matmul.
PV: out^T[64, 512] += V^T_kk @ P^T_kk accumulated in PSUM over kk.
out^T goes back in [D, S] layout (fp16 by default: po <= ~1.4e3, well
inside fp16, halves output DMA); the host transposes and applies
D_q * e^{-C}. All O(S^2) work stays on device.

I/O hygiene (rev 2026-08-08): V is pre-transposed on host to
[128, (S/128)*D] per head so the device DMA is partition-contiguous
(was a (t p) d -> p t d DRAM gather = ~10k 128B descriptors clogging
all 16 queues, delaying output drain by ~12us). Warmup weights come
from a VectorE memset instead of the mask DMA so PE warmup issues
~2.5us earlier. Keep PAD_W=512: thinner pads (256) weaken the HAM
bridge and cost 10-30us. Chip power-throttling (P0) ~60us into a run
adds 10-25% run-to-run variance; compare medians across runs.
"""

import os
import sys

if "/opt/trn_rl_repo" not in sys.path:
    sys.path.insert(0, "/opt/trn_rl_repo")

import numpy as np

B, H, S, D = 2, 16, 2048, 64
SM = 0.125
N_CORES = 8
HPC = (B * H) // N_CORES  # heads per core = 4
SPAN = 512  # query-span per PSUM accumulation group
NSPAN = S // SPAN  # 4
KTILE = 128  # key rows per logits tile
EXP_SCALE = 2.0 * SM  # 0.25

# knobs
DT = os.environ.get("KRN_DT", "f16")  # "f16" | "f32r"
GROUP = int(os.environ.get("KRN_GROUP", "3"))  # logits tiles per ACTIVATE
LG_BUFS = int(os.environ.get("KRN_LG_BUFS", "2"))  # logits psum buffers
TRIM = os.environ.get("KRN_TRIM", "0") == "1"  # diag-tile column trim
WARMUP = int(os.environ.get("KRN_WARMUP", "20"))  # PE warmup matmuls
DTRIM = os.environ.get("KRN_DTRIM", "1") == "1"  # skip dead col-prefix of diag tiles
ALT = os.environ.get("KRN_ALT", "0") == "1"  # alternating 4+2 logits psum pools
PAD_GROUPS = int(os.environ.get("KRN_PAD_GROUPS", "8"))  # early groups padded
PAD_N = int(os.environ.get("KRN_PAD_N", "3"))  # dummy matmuls per padded group
PAD_W = int(os.environ.get("KRN_PAD_W", "512"))  # pad/warmup matmul stream width
PT_BUFS = int(os.environ.get("KRN_PT_BUFS", "6"))
OT16 = os.environ.get("KRN_OT16", "1") == "1"  # fp16 output DMA

_CACHE = {}


def _build_module():
    """Build + compile the Bass module (once per process per variant)."""
    key = ("nc", DT, GROUP, LG_BUFS, TRIM, WARMUP, PT_BUFS, DTRIM, ALT, PAD_GROUPS, PAD_N, PAD_W, OT16)
    if key in _CACHE:
        return _CACHE[key]

    import concourse.mybir as mybir
    import concourse.tile as tile
    from concourse import bacc

    f32 = mybir.dt.float32
    mmdt = mybir.dt.float16 if DT == "f16" else mybir.dt.float32r
    naug = 2 if DT == "f16" else 0

    nc = bacc.Bacc(
        "TRN2", target_bir_lowering=False, debug=False, num_devices=N_CORES
    )

    otdt = mybir.dt.float16 if OT16 else f32
    qT = nc.dram_tensor("qT", [HPC, D + naug, S], mmdt, kind="ExternalInput").ap()
    kT = nc.dram_tensor("kT", [HPC, D + naug, S], mmdt, kind="ExternalInput").ap()
    # v pre-transposed on host to [128, S//128 * D] per head: contiguous DMA
    v = nc.dram_tensor("v", [HPC, 128, (S // 128) * D], mmdt, kind="ExternalInput").ap()
    maskc = nc.dram_tensor("maskc", [128, 896], mmdt, kind="ExternalInput").ap()
    biasc = nc.dram_tensor("biasc", [128, 1], f32, kind="ExternalInput").ap()
    ot = nc.dram_tensor("ot", [HPC, D, S], otdt, kind="ExternalOutput").ap()

    with tile.TileContext(nc) as tc:
        with (
            tc.tile_pool(name="consts", bufs=1) as consts,
            tc.tile_pool(name="qk_sb", bufs=2) as qk_sb,
            tc.tile_pool(name="v_sb", bufs=2) as v_sb,
            tc.tile_pool(name="pt_sb", bufs=PT_BUFS) as pt_sb,
            tc.tile_pool(name="ot_sb", bufs=2) as ot_sb,
            tc.tile_pool(name="lg_ps", bufs=(1 if ALT else LG_BUFS), space="PSUM") as lg_ps,
            tc.tile_pool(name="lgb_ps", bufs=1, space="PSUM") as lgb_ps,
            tc.tile_pool(name="pv_ps", bufs=2, space="PSUM") as pv_ps,
        ):
            # warmup weights via memset (no DMA dependency): PE warm-up can
            # start the moment the sequencers come up, before input DMAs land
            warmsb = consts.tile([128, PAD_W], mmdt, tag="warm")
            nc.vector.memset(warmsb, 0.0)

            masksb = consts.tile([128, 896], mmdt, tag="mask")
            nc.sync.dma_start(out=masksb, in_=maskc)
            biassb = consts.tile([128, 1], f32, tag="bias")
            nc.sync.dma_start(out=biassb, in_=biasc)

            # dense dummy matmuls at start: trip the PE HAM clock-gate to
            # K=8/8 and bridge seamlessly into the first real matmuls.
            # Must be full-128-partition matmuls: partial-array (row_grp)
            # work does not register as PE-busy for the clock gate.
            for w in range(WARMUP):
                wps = pv_ps.tile([D, SPAN], f32, tag="po")
                nc.tensor.matmul(
                    wps[:, 0:PAD_W],
                    warmsb[:, 0:D],
                    warmsb[:, 0:PAD_W],
                    start=True,
                    stop=True,
                )

            def emit_pv(pend):
                # PV matmuls for a completed exp group (trails the logits of
                # the next group so the in-order PE stream never stalls on ACT)
                po_, pt_, gkk_, nkk_, s_, vsb_, h_ = pend
                ndiag = SPAN // KTILE
                for j, kk in enumerate(gkk_):
                    jd = kk - s_ * ndiag
                    first, last = kk == 0, kk == nkk_ - 1
                    if TRIM and jd >= 0:
                        # corner block [128jd, 128jd+128) masked; cols beyond
                        # fully valid; cols before fully masked -> skipped
                        c1 = (jd + 1) * KTILE
                        if c1 < SPAN:
                            # start=True pends-zero the whole 2KB bank, so
                            # only the first emitted piece may carry it
                            nc.tensor.matmul(
                                po_[:, c1:SPAN],
                                vsb_[:, kk, :],
                                pt_[:, j, c1:SPAN],
                                start=first,
                                stop=False,
                            )
                        nc.tensor.matmul(
                            po_[:, jd * KTILE : c1],
                            vsb_[:, kk, :],
                            pt_[:, j, jd * KTILE : c1],
                            start=first and c1 >= SPAN,
                            stop=last,
                        )
                    else:
                        c0 = jd * KTILE if (DTRIM and jd > 0) else 0
                        nc.tensor.matmul(
                            po_[:, c0:SPAN],
                            vsb_[:, kk, :],
                            pt_[:, j, c0:SPAN],
                            start=first,
                            stop=last,
                        )
                if gkk_[-1] == nkk_ - 1:  # span finished
                    oT = ot_sb.tile([D, SPAN], mmdt if OT16 else f32, tag="oT")
                    nc.vector.tensor_copy(oT, po_)
                    nc.sync.dma_start(
                        out=ot[h_, :, s_ * SPAN : (s_ + 1) * SPAN], in_=oT
                    )

            pending = None
            alt_par = [0]
            gcount = [0]
            for h in range(HPC):
                qta = qk_sb.tile([D + naug, S], mmdt, tag="qta")
                kta = qk_sb.tile([D + naug, S], mmdt, tag="kta")
                nc.sync.dma_start(out=qta, in_=qT[h])
                nc.sync.dma_start(out=kta, in_=kT[h])

                vsb = v_sb.tile([128, S // 128, D], mmdt, tag="vsb")
                nc.sync.dma_start(
                    out=vsb, in_=v[h].rearrange("p (t d) -> p t d", d=D)
                )

                for s in range(NSPAN):
                    ndiag = SPAN // KTILE
                    nkk = (s + 1) * ndiag  # causal: key tiles 0..nkk-1
                    po = pv_ps.tile([D, SPAN], f32, tag="po")
                    qspan = qta[:, s * SPAN : (s + 1) * SPAN]
                    # chunk the kk list; with ALT, alternate between a 4-bank
                    # and a 2-bank PSUM tile so ACTIVATE count drops while
                    # PE/ACT still double-buffer across the two pools
                    chunks = []
                    g0 = 0
                    while g0 < nkk:
                        if ALT:
                            size = 4 if alt_par[0] == 0 else 2
                            alt_par[0] ^= 1
                        else:
                            size = GROUP
                        chunks.append(list(range(g0, min(g0 + size, nkk))))
                        g0 += size
                    for gkk in chunks:
                        n = len(gkk)
                        if ALT:
                            if len(gkk) > 2:
                                pl = lg_ps.tile([128, 4, SPAN], f32, tag="pl")
                            else:
                                pl = lgb_ps.tile([128, 2, SPAN], f32, tag="plb")
                        else:
                            pl = lg_ps.tile([128, GROUP, SPAN], f32, tag="pl")
                        gjd0 = gkk[0] - s * ndiag
                        gc0 = gjd0 * KTILE if (DTRIM and gjd0 > 0) else 0
                        for j, kk in enumerate(gkk):
                            # trim only to the group-common dead prefix so the
                            # grouped exp below reads fully-written PSUM
                            nc.tensor.matmul(
                                pl[:, j, gc0:SPAN],
                                kta[:, kk * KTILE : (kk + 1) * KTILE],
                                qspan[:, gc0:SPAN],
                                start=True,
                                stop=True,
                            )
                        if pending is not None:
                            emit_pv(pending)
                            pending = None
                        if h == 0 and gcount[0] < PAD_GROUPS:
                            # keep the PE busy-window saturated through the
                            # pipeline-fill phase so the HAM clock-gate never
                            # sees an idle window and re-throttles
                            gcount[0] += 1
                            for _ in range(PAD_N):
                                wps = pv_ps.tile([D, SPAN], f32, tag="po")
                                nc.tensor.matmul(
                                    wps[:, 0:PAD_W],
                                    warmsb[:, 0:D],
                                    warmsb[:, 0:PAD_W],
                                    start=True,
                                    stop=True,
                                )
                        pt = pt_sb.tile(
                            [128, max(GROUP, 4 if ALT else 0), SPAN], mmdt, tag="pt"
                        )
                        jd0 = gkk[0] - s * ndiag
                        ec0 = jd0 * KTILE if (DTRIM and jd0 > 0) else 0
                        nc.scalar.activation(
                            pt[:, 0:n, ec0:SPAN],
                            pl[:, 0:n, ec0:SPAN],
                            mybir.ActivationFunctionType.Exp,
                            bias=biassb,
                            scale=EXP_SCALE,
                        )
                        for j, kk in enumerate(gkk):
                            jd = kk - s * ndiag
                            if jd >= 0:  # diagonal tile -> causal mask
                                if TRIM:
                                    # mask only the triangular corner block
                                    nc.vector.tensor_mul(
                                        pt[:, j, jd * KTILE : (jd + 1) * KTILE],
                                        pt[:, j, jd * KTILE : (jd + 1) * KTILE],
                                        masksb[:, 384:512],
                                    )
                                elif DTRIM:
                                    mc0 = jd * KTILE
                                    nc.vector.tensor_mul(
                                        pt[:, j, mc0:SPAN],
                                        pt[:, j, mc0:SPAN],
                                        masksb[:, 384 : 896 - mc0],
                                    )
                                else:
                                    c0 = 384 - 128 * jd
                                    nc.vector.tensor_mul(
                                        pt[:, j, :],
                                        pt[:, j, :],
                                        masksb[:, c0 : c0 + SPAN],
                                    )
                        pending = (po, pt, gkk, nkk, s, vsb, h)
            if pending is not None:
                emit_pv(pending)

    nc.compile()
    _CACHE[key] = nc
    return nc


def _host_prep(q, k, v):
    """Shard + relayout inputs for the 8 cores. Returns (in_maps, row_scale)."""
    q = np.ascontiguousarray(np.asarray(q, dtype=np.float32)).reshape(B * H, S, D)
    k = np.ascontiguousarray(np.asarray(k, dtype=np.float32)).reshape(B * H, S, D)
    v = np.ascontiguousarray(np.asarray(v, dtype=np.float32)).reshape(B * H, S, D)

    qsq = (q.astype(np.float32) ** 2).sum(-1)  # [BH, S]
    ksq = (k.astype(np.float32) ** 2).sum(-1)

    if DT == "f16":
        npdt = np.float16
        # pt <= e^{0.125*max(qsq) + C}; keep under ~e^{10.5} (fp16 max 65504)
        C = float(min(10.5 - SM * qsq.max(), 0.0))
        qT = np.zeros((B * H, D + 2, S), np.float16)
        kT = np.zeros((B * H, D + 2, S), np.float16)
        qT[:, :D, :] = q.transpose(0, 2, 1)
        kT[:, :D, :] = k.transpose(0, 2, 1)
        qT[:, D, :] = -0.5
        qT[:, D + 1, :] = -0.5
        khi = ksq.astype(np.float16)
        klo = (ksq - khi.astype(np.float32)).astype(np.float16)
        kT[:, D, :] = khi
        kT[:, D + 1, :] = klo
        vin = v.astype(np.float16)
        # device layout [BH, 128, (S//128)*D]: partition-contiguous V tiles
        vin = np.ascontiguousarray(
            vin.reshape(B * H, S // 128, 128, D).transpose(0, 2, 1, 3)
        ).reshape(B * H, 128, (S // 128) * D)
        # host applies D_q * e^{-C}
        row_scale = np.exp(-SM * qsq.astype(np.float64) - C).astype(np.float32)
    else:
        npdt = np.float32
        C = 0.0
        qT = np.ascontiguousarray(q.transpose(0, 2, 1))
        kT = np.ascontiguousarray(k.transpose(0, 2, 1))
        dk = np.exp(-SM * ksq.astype(np.float64)).astype(np.float32)
        vin = v * dk[:, :, None]  # V' = D_k V
        vin = np.ascontiguousarray(
            vin.reshape(B * H, S // 128, 128, D).transpose(0, 2, 1, 3)
        ).reshape(B * H, 128, (S // 128) * D)
        row_scale = np.exp(-SM * qsq.astype(np.float64)).astype(np.float32)

    # maskc[r, c] = 1 if c >= r + 384 else 0 ; slice [384-128j : 896-128j]
    # gives the causal mask for diagonal tile offset j; [384:512] is the
    # corner-block mask (q_local >= k_local)
    r = np.arange(128)[:, None]
    c = np.arange(896)[None, :]
    maskc = (c >= r + 384).astype(npdt)
    biasc = np.full((128, 1), C, dtype=np.float32)

    in_maps = []
    for core in range(N_CORES):
        sl = slice(core * HPC, (core + 1) * HPC)
        in_maps.append(
            {
                "qT": np.ascontiguousarray(qT[sl]),
                "kT": np.ascontiguousarray(kT[sl]),
                "v": np.ascontiguousarray(vin[sl]),
                "maskc": maskc,
                "biasc": biasc,
            }
        )
    return in_maps, row_scale


def _gather(results, row_scale):
    """results[core]["ot"] : [HPC, D, S] -> full [B, H, S, D] (applies D_q)."""
    outs = [np.asarray(r["ot"]) for r in results]
    o = np.concatenate(outs, axis=0)  # [BH, D, S]
    o = o.transpose(0, 2, 1) * row_scale[:, :, None]  # [BH, S, D]
    return np.ascontiguousarray(o.reshape(B, H, S, D).astype(np.float32))


def kernel(q, k, v):
    from concourse.bass_utils import run_bass_kernel_spmd

    nc = _build_module()
    in_maps, row_scale = _host_prep(q, k, v)
    res = run_bass_kernel_spmd(nc, in_maps, core_ids=list(range(N_CORES)))
    return _gather(res.results, row_scale)


if __name__ == "__main__":
    rng = np.random.default_rng(0)
    q = rng.standard_normal((B, H, S, D), dtype=np.float32)
    k = rng.standard_normal((B, H, S, D), dtype=np.float32)
    v = rng.standard_normal((B, H, S, D), dtype=np.float32)
    o = kernel(q, k, v)
    print("out", o.shape, o.dtype, float(np.abs(o).max()))



# revision 19
# speedup vs baseline: 1.1677x; 1.1677x over previous
"""Trainium2 Bass kernel for causal RBF (squared-exponential) attention.

  p_ij = exp(-sm * ||q_i - k_j||^2) causal-masked, out = p @ v (no normalization)
  B,H,S,D = 2,16,2048,64 ; sm = 0.125

Sharding: B*H = 32 heads, 4 heads per core across 8 NeuronCores (head
parallel, no cross-core comm).

Factorization: p = D_q . exp(2*sm*qk - sm*||k||^2) with D_q diagonal in
exp(-sm*||q||^2). D_q is applied to the output rows on the host (O(N));
the device computes, per 128-key x 512-query tile, one fp16 matmul with a
66-deep contraction (rows 0..63 = K^T/Q^T, rows 64/65 = ksq split hi/lo
against -1/2 const rows), then ScalarE evaluates
    pt = exp(0.25 * psum + C)       (psum = qk - ksq/2)
straight PSUM -> SBUF. C is a host-supplied bias chosen from max(qsq) so
that pt <= e^{0.125 qsq + C} stays inside fp16 range (p <= 1 identity).
Diagonal tiles: fully-masked columns are skipped in PV entirely; only the
128x128 triangular corner is masked (VectorE) before the PV matmul.
PV: out^T[64, 512] += V^T_kk @ P^T_kk accumulated in PSUM over kk.
out^T goes back in [D, S] layout (fp16 by default: po <= ~1.4e3, well
inside fp16, halves output DMA); the host transposes and applies
D_q * e^{-C}. All O(S^2) work stays on device.

I/O hygiene (rev 2026-08-08): V is pre-transposed on host to
[128, (S/128)*D] per head so the device DMA is partition-contiguous
(was a (t p) d -> p t d DRAM gather = ~10k 128B descriptors clogging
all 16 queues, delaying output drain by ~12us). Warmup weights come
from a VectorE memset instead of the mask DMA so PE warmup issues
~2.5us earlier. Keep PAD_W=512: thinner pads (256) weaken the HAM
bridge and cost 10-30us. Chip power-throttling (P0) ~60us into a run
adds 10-25% run-to-run variance; compare medians across runs.
"""

import os
import sys

if "/opt/trn_rl_repo" not in sys.path:
    sys.path.insert(0, "/opt/trn_rl_repo")

import numpy as np

B, H, S, D = 2, 16, 2048, 64
SM = 0.125
N_CORES = 8
HPC = (B * H) // N_CORES  # heads per core = 4
SPAN = 512  # query-span per PSUM accumulation group
NSPAN = S // SPAN  # 4
KTILE = 128  # key rows per logits tile
EXP_SCALE = 2.0 * SM  # 0.25

# knobs
DT = os.environ.get("KRN_DT", "f16")  # "f16" | "f32r"
GROUP = int(os.environ.get("KRN_GROUP", "3"))  # logits tiles per ACTIVATE
LG_BUFS = int(os.environ.get("KRN_LG_BUFS", "2"))  # logits psum buffers
TRIM = os.environ.get("KRN_TRIM", "0") == "1"  # diag-tile column trim
WARMUP = int(os.environ.get("KRN_WARMUP", "20"))  # PE warmup matmuls
DTRIM = os.environ.get("KRN_DTRIM", "1") == "1"  # skip dead col-prefix of diag tiles
ALT = os.environ.get("KRN_ALT", "0") == "1"  # alternating 4+2 logits psum pools
PAD_GROUPS = int(os.environ.get("KRN_PAD_GROUPS", "8"))  # early groups padded
PAD_N = int(os.environ.get("KRN_PAD_N", "3"))  # dummy matmuls per padded group
PAD_W = int(os.environ.get("KRN_PAD_W", "512"))  # pad/warmup matmul stream width
PT_BUFS = int(os.environ.get("KRN_PT_BUFS", "6"))
OT16 = os.environ.get("KRN_OT16", "1") == "1"  # fp16 output DMA
PV_BUFS = int(os.environ.get("KRN_PV_BUFS", "1"))  # po psum buffers
TAILPOOL = os.environ.get("KRN_TAILPOOL", "1") == "1"  # 1-bank pool for size-1 groups+pads

_CACHE = {}


def _build_module():
    """Build + compile the Bass module (once per process per variant)."""
    key = ("nc", DT, GROUP, LG_BUFS, TRIM, WARMUP, PT_BUFS, DTRIM, ALT, PAD_GROUPS, PAD_N, PAD_W, OT16, PV_BUFS, TAILPOOL)
    if key in _CACHE:
        return _CACHE[key]

    import concourse.mybir as mybir
    import concourse.tile as tile
    from concourse import bacc

    f32 = mybir.dt.float32
    mmdt = mybir.dt.float16 if DT == "f16" else mybir.dt.float32r
    naug = 2 if DT == "f16" else 0

    nc = bacc.Bacc(
        "TRN2", target_bir_lowering=False, debug=False, num_devices=N_CORES
    )

    otdt = mybir.dt.float16 if OT16 else f32
    qT = nc.dram_tensor("qT", [HPC, D + naug, S], mmdt, kind="ExternalInput").ap()
    kT = nc.dram_tensor("kT", [HPC, D + naug, S], mmdt, kind="ExternalInput").ap()
    # v pre-transposed on host to [128, S//128 * D] per head: contiguous DMA
    v = nc.dram_tensor("v", [HPC, 128, (S // 128) * D], mmdt, kind="ExternalInput").ap()
    maskc = nc.dram_tensor("maskc", [128, 896], mmdt, kind="ExternalInput").ap()
    biasc = nc.dram_tensor("biasc", [128, 1], f32, kind="ExternalInput").ap()
    ot = nc.dram_tensor("ot", [HPC, D, S], otdt, kind="ExternalOutput").ap()

    with tile.TileContext(nc) as tc:
        with (
            tc.tile_pool(name="consts", bufs=1) as consts,
            tc.tile_pool(name="qk_sb", bufs=2) as qk_sb,
            tc.tile_pool(name="v_sb", bufs=2) as v_sb,
            tc.tile_pool(name="pt_sb", bufs=PT_BUFS) as pt_sb,
            tc.tile_pool(name="ot_sb", bufs=2) as ot_sb,
            tc.tile_pool(name="lg_ps", bufs=(1 if ALT else LG_BUFS), space="PSUM") as lg_ps,
            tc.tile_pool(name="lgb_ps", bufs=1, space="PSUM") as lgb_ps,
            tc.tile_pool(name="pv_ps", bufs=(2 if ALT or not TAILPOOL else PV_BUFS), space="PSUM") as pv_ps,
        ):
            # warmup weights via memset (no DMA dependency): PE warm-up can
            # start the moment the sequencers come up, before input DMAs land
            warmsb = consts.tile([128, PAD_W], mmdt, tag="warm")
            nc.vector.memset(warmsb, 0.0)

            masksb = consts.tile([128, 896], mmdt, tag="mask")
            nc.sync.dma_start(out=masksb, in_=maskc)
            biassb = consts.tile([128, 1], f32, tag="bias")
            nc.sync.dma_start(out=biassb, in_=biasc)

            # dense dummy matmuls at start: trip the PE HAM clock-gate to
            # K=8/8 and bridge seamlessly into the first real matmuls.
            # Must be full-128-partition matmuls: partial-array (row_grp)
            # work does not register as PE-busy for the clock gate.
            def pad_mm():
                if TAILPOOL:
                    # dummy MMs live in the 1-bank tail pool so they never
                    # collide with a live po accumulation (pv_ps bufs=1)
                    tl = lgb_ps.tile([128, 1, SPAN], f32, tag="tail")
                    nc.tensor.matmul(
                        tl[:, 0, 0:PAD_W],
                        warmsb[:, 0:128],
                        warmsb[:, 0:PAD_W],
                        start=True,
                        stop=True,
                    )
                else:
                    wps = pv_ps.tile([D, SPAN], f32, tag="po")
                    nc.tensor.matmul(
                        wps[:, 0:PAD_W],
                        warmsb[:, 0:D],
                        warmsb[:, 0:PAD_W],
                        start=True,
                        stop=True,
                    )

            for w in range(WARMUP):
                pad_mm()

            def emit_pv(pend):
                # PV matmuls for a completed exp group (trails the logits of
                # the next group so the in-order PE stream never stalls on ACT)
                po_, pt_, gkk_, nkk_, s_, vsb_, h_ = pend
                ndiag = SPAN // KTILE
                for j, kk in enumerate(gkk_):
                    jd = kk - s_ * ndiag
                    first, last = kk == 0, kk == nkk_ - 1
                    if TRIM and jd >= 0:
                        # corner block [128jd, 128jd+128) masked; cols beyond
                        # fully valid; cols before fully masked -> skipped
                        c1 = (jd + 1) * KTILE
                        if c1 < SPAN:
                            # start=True pends-zero the whole 2KB bank, so
                            # only the first emitted piece may carry it
                            nc.tensor.matmul(
                                po_[:, c1:SPAN],
                                vsb_[:, kk, :],
                                pt_[:, j, c1:SPAN],
                                start=first,
                                stop=False,
                            )
                        nc.tensor.matmul(
                            po_[:, jd * KTILE : c1],
                            vsb_[:, kk, :],
                            pt_[:, j, jd * KTILE : c1],
                            start=first and c1 >= SPAN,
                            stop=last,
                        )
                    else:
                        c0 = jd * KTILE if (DTRIM and jd > 0) else 0
                        nc.tensor.matmul(
                            po_[:, c0:SPAN],
                            vsb_[:, kk, :],
                            pt_[:, j, c0:SPAN],
                            start=first,
                            stop=last,
                        )
                if gkk_[-1] == nkk_ - 1:  # span finished
                    oT = ot_sb.tile([D, SPAN], mmdt if OT16 else f32, tag="oT")
                    nc.vector.tensor_copy(oT, po_)
                    nc.sync.dma_start(
                        out=ot[h_, :, s_ * SPAN : (s_ + 1) * SPAN], in_=oT
                    )

            pending = None
            alt_par = [0]
            gcount = [0]
            for h in range(HPC):
                qta = qk_sb.tile([D + naug, S], mmdt, tag="qta")
                kta = qk_sb.tile([D + naug, S], mmdt, tag="kta")
                nc.sync.dma_start(out=qta, in_=qT[h])
                nc.sync.dma_start(out=kta, in_=kT[h])

                vsb = v_sb.tile([128, S // 128, D], mmdt, tag="vsb")
                nc.sync.dma_start(
                    out=vsb, in_=v[h].rearrange("p (t d) -> p t d", d=D)
                )

                for s in range(NSPAN):
                    ndiag = SPAN // KTILE
                    nkk = (s + 1) * ndiag  # causal: key tiles 0..nkk-1
                    po = pv_ps.tile([D, SPAN], f32, tag="po")
                    qspan = qta[:, s * SPAN : (s + 1) * SPAN]
                    # chunk the kk list; with ALT, alternate between a 4-bank
                    # and a 2-bank PSUM tile so ACTIVATE count drops while
                    # PE/ACT still double-buffer across the two pools
                    chunks = []
                    g0 = 0
                    while g0 < nkk:
                        if ALT:
                            size = 4 if alt_par[0] == 0 else 2
                            alt_par[0] ^= 1
                        else:
                            size = GROUP
                        chunks.append(list(range(g0, min(g0 + size, nkk))))
                        g0 += size
                    for gkk in chunks:
                        n = len(gkk)
                        if ALT:
                            if len(gkk) > 2:
                                pl = lg_ps.tile([128, 4, SPAN], f32, tag="pl")
                            else:
                                pl = lgb_ps.tile([128, 2, SPAN], f32, tag="plb")
                        elif TAILPOOL and len(gkk) == 1:
                            # size-1 span-tail group: own bank, keeps the main
                            # 2-buf pl rotation free for the next span's QK
                            pl = lgb_ps.tile([128, 1, SPAN], f32, tag="tail")
                        else:
                            pl = lg_ps.tile([128, GROUP, SPAN], f32, tag="pl")
                        gjd0 = gkk[0] - s * ndiag
                        gc0 = gjd0 * KTILE if (DTRIM and gjd0 > 0) else 0
                        for j, kk in enumerate(gkk):
                            # trim only to the group-common dead prefix so the
                            # grouped exp below reads fully-written PSUM
                            nc.tensor.matmul(
                                pl[:, j, gc0:SPAN],
                                kta[:, kk * KTILE : (kk + 1) * KTILE],
                                qspan[:, gc0:SPAN],
                                start=True,
                                stop=True,
                            )
                        if pending is not None:
                            emit_pv(pending)
                            pending = None
                        if h == 0 and gcount[0] < PAD_GROUPS:
                            # keep the PE busy-window saturated through the
                            # pipeline-fill phase so the HAM clock-gate never
                            # sees an idle window and re-throttles
                            gcount[0] += 1
                            for _ in range(PAD_N):
                                pad_mm()
                        pt = pt_sb.tile(
                            [128, max(GROUP, 4 if ALT else 0), SPAN], mmdt, tag="pt"
                        )
                        jd0 = gkk[0] - s * ndiag
                        ec0 = jd0 * KTILE if (DTRIM and jd0 > 0) else 0
                        nc.scalar.activation(
                            pt[:, 0:n, ec0:SPAN],
                            pl[:, 0:n, ec0:SPAN],
                            mybir.ActivationFunctionType.Exp,
                            bias=biassb,
                            scale=EXP_SCALE,
                        )
                        for j, kk in enumerate(gkk):
                            jd = kk - s * ndiag
                            if jd >= 0:  # diagonal tile -> causal mask
                                if TRIM:
                                    # mask only the triangular corner block
                                    nc.vector.tensor_mul(
                                        pt[:, j, jd * KTILE : (jd + 1) * KTILE],
                                        pt[:, j, jd * KTILE : (jd + 1) * KTILE],
                                        masksb[:, 384:512],
                                    )
                                elif DTRIM:
                                    mc0 = jd * KTILE
                                    nc.vector.tensor_mul(
                                        pt[:, j, mc0:SPAN],
                                        pt[:, j, mc0:SPAN],
                                        masksb[:, 384 : 896 - mc0],
                                    )
                                else:
                                    c0 = 384 - 128 * jd
                                    nc.vector.tensor_mul(
                                        pt[:, j, :],
                                        pt[:, j, :],
                                        masksb[:, c0 : c0 + SPAN],
                                    )
                        pending = (po, pt, gkk, nkk, s, vsb, h)
            if pending is not None:
                emit_pv(pending)

    nc.compile()
    _CACHE[key] = nc
    return nc


def _host_prep(q, k, v):
    """Shard + relayout inputs for the 8 cores. Returns (in_maps, row_scale)."""
    q = np.ascontiguousarray(np.asarray(q, dtype=np.float32)).reshape(B * H, S, D)
    k = np.ascontiguousarray(np.asarray(k, dtype=np.float32)).reshape(B * H, S, D)
    v = np.ascontiguousarray(np.asarray(v, dtype=np.float32)).reshape(B * H, S, D)

    qsq = (q.astype(np.float32) ** 2).sum(-1)  # [BH, S]
    ksq = (k.astype(np.float32) ** 2).sum(-1)

    if DT == "f16":
        npdt = np.float16
        # pt <= e^{0.125*max(qsq) + C}; keep under ~e^{10.5} (fp16 max 65504)
        C = float(min(10.5 - SM * qsq.max(), 0.0))
        qT = np.zeros((B * H, D + 2, S), np.float16)
        kT = np.zeros((B * H, D + 2, S), np.float16)
        qT[:, :D, :] = q.transpose(0, 2, 1)
        kT[:, :D, :] = k.transpose(0, 2, 1)
        qT[:, D, :] = -0.5
        qT[:, D + 1, :] = -0.5
        khi = ksq.astype(np.float16)
        klo = (ksq - khi.astype(np.float32)).astype(np.float16)
        kT[:, D, :] = khi
        kT[:, D + 1, :] = klo
        vin = v.astype(np.float16)
        # device layout [BH, 128, (S//128)*D]: partition-contiguous V tiles
        vin = np.ascontiguousarray(
            vin.reshape(B * H, S // 128, 128, D).transpose(0, 2, 1, 3)
        ).reshape(B * H, 128, (S // 128) * D)
        # host applies D_q * e^{-C}
        row_scale = np.exp(-SM * qsq.astype(np.float64) - C).astype(np.float32)
    else:
        npdt = np.float32
        C = 0.0
        qT = np.ascontiguousarray(q.transpose(0, 2, 1))
        kT = np.ascontiguousarray(k.transpose(0, 2, 1))
        dk = np.exp(-SM * ksq.astype(np.float64)).astype(np.float32)
        vin = v * dk[:, :, None]  # V' = D_k V
        vin = np.ascontiguousarray(
            vin.reshape(B * H, S // 128, 128, D).transpose(0, 2, 1, 3)
        ).reshape(B * H, 128, (S // 128) * D)
        row_scale = np.exp(-SM * qsq.astype(np.float64)).astype(np.float32)

    # maskc[r, c] = 1 if c >= r + 384 else 0 ; slice [384-128j : 896-128j]
    # gives the causal mask for diagonal tile offset j; [384:512] is the
    # corner-block mask (q_local >= k_local)
    r = np.arange(128)[:, None]
    c = np.arange(896)[None, :]
    maskc = (c >= r + 384).astype(npdt)
    biasc = np.full((128, 1), C, dtype=np.float32)

    in_maps = []
    for core in range(N_CORES):
        sl = slice(core * HPC, (core + 1) * HPC)
        in_maps.append(
            {
                "qT": np.ascontiguousarray(qT[sl]),
                "kT": np.ascontiguousarray(kT[sl]),
                "v": np.ascontiguousarray(vin[sl]),
                "maskc": maskc,
                "biasc": biasc,
            }
        )
    return in_maps, row_scale


def _gather(results, row_scale):
    """results[core]["ot"] : [HPC, D, S] -> full [B, H, S, D] (applies D_q)."""
    outs = [np.asarray(r["ot"]) for r in results]
    o = np.concatenate(outs, axis=0)  # [BH, D, S]
    o = o.transpose(0, 2, 1) * row_scale[:, :, None]  # [BH, S, D]
    return np.ascontiguousarray(o.reshape(B, H, S, D).astype(np.float32))


def kernel(q, k, v):
    from concourse.bass_utils import run_bass_kernel_spmd

    nc = _build_module()
    in_maps, row_scale = _host_prep(q, k, v)
    res = run_bass_kernel_spmd(nc, in_maps, core_ids=list(range(N_CORES)))
    return _gather(res.results, row_scale)


if __name__ == "__main__":
    rng = np.random.default_rng(0)
    q = rng.standard_normal((B, H, S, D), dtype=np.float32)
    k = rng.standard_normal((B, H, S, D), dtype=np.float32)
    v = rng.standard_normal((B, H, S, D), dtype=np.float32)
    o = kernel(q, k, v)
    print("out", o.shape, o.dtype, float(np.abs(o).max()))



# revision 24
# speedup vs baseline: 1.2670x; 1.0850x over previous
"""Trainium2 Bass kernel for causal RBF (squared-exponential) attention.

  p_ij = exp(-sm * ||q_i - k_j||^2) causal-masked, out = p @ v (no normalization)
  B,H,S,D = 2,16,2048,64 ; sm = 0.125

Sharding: B*H = 32 heads, 4 heads per core across 8 NeuronCores (head
parallel, no cross-core comm).

Factorization: p = D_q . exp(2*sm*qk - sm*||k||^2) with D_q diagonal in
exp(-sm*||q||^2). D_q is applied to the output rows on the host (O(N));
the device computes, per 128-key x 512-query tile, one fp16 matmul with a
66-deep contraction (rows 0..63 = K^T/Q^T, rows 64/65 = ksq split hi/lo
against -1/2 const rows), then ScalarE evaluates
    pt = exp(0.25 * psum + C)       (psum = qk - ksq/2)
straight PSUM -> SBUF. C is a host-supplied bias chosen from max(qsq) so
that pt <= e^{0.125 qsq + C} stays inside fp16 range (p <= 1 identity).
Diagonal tiles: fully-masked columns are skipped in PV entirely; only the
128x128 triangular corner is masked (VectorE) before the PV matmul.
PV: out^T[64, 512] += V^T_kk @ P^T_kk accumulated in PSUM over kk.
out^T goes back in [D, S] layout (fp16 by default: po <= ~1.4e3, well
inside fp16, halves output DMA); the host transposes and applies
D_q * e^{-C}. All O(S^2) work stays on device.

I/O hygiene (rev 2026-08-08): V is pre-transposed on host to
[128, (S/128)*D] per head so the device DMA is partition-contiguous
(was a (t p) d -> p t d DRAM gather = ~10k 128B descriptors clogging
all 16 queues, delaying output drain by ~12us). Warmup weights come
from a VectorE memset instead of the mask DMA so PE warmup issues
~2.5us earlier. Keep PAD_W=512: thinner pads (256) weaken the HAM
bridge and cost 10-30us. Chip power-throttling (P0) ~60us into a run
adds 10-25% run-to-run variance; compare medians across runs.
"""

import os
import sys

if "/opt/trn_rl_repo" not in sys.path:
    sys.path.insert(0, "/opt/trn_rl_repo")

import numpy as np

B, H, S, D = 2, 16, 2048, 64
SM = 0.125
N_CORES = 8
HPC = (B * H) // N_CORES  # heads per core = 4
SPAN = 512  # query-span per PSUM accumulation group
NSPAN = S // SPAN  # 4
KTILE = 128  # key rows per logits tile
EXP_SCALE = 2.0 * SM  # 0.25

# knobs
DT = os.environ.get("KRN_DT", "f16")  # "f16" | "f32r"
GROUP = int(os.environ.get("KRN_GROUP", "3"))  # logits tiles per ACTIVATE
LG_BUFS = int(os.environ.get("KRN_LG_BUFS", "2"))  # logits psum buffers
TRIM = os.environ.get("KRN_TRIM", "0") == "1"  # diag-tile column trim
WARMUP = int(os.environ.get("KRN_WARMUP", "20"))  # PE warmup matmuls
DTRIM = os.environ.get("KRN_DTRIM", "1") == "1"  # skip dead col-prefix of diag tiles
ALT = os.environ.get("KRN_ALT", "0") == "1"  # alternating 4+2 logits psum pools
PAD_GROUPS = int(os.environ.get("KRN_PAD_GROUPS", "8"))  # early groups padded
PAD_N = int(os.environ.get("KRN_PAD_N", "3"))  # dummy matmuls per padded group
PAD_W = int(os.environ.get("KRN_PAD_W", "512"))  # pad/warmup matmul stream width
PT_BUFS = int(os.environ.get("KRN_PT_BUFS", "6"))
OT16 = os.environ.get("KRN_OT16", "1") == "1"  # fp16 output DMA
PV_BUFS = int(os.environ.get("KRN_PV_BUFS", "1"))  # po psum buffers
TAILPOOL = os.environ.get("KRN_TAILPOOL", "1") == "1"  # 1-bank pool for size-1 groups+pads
PV_LAG = int(os.environ.get("KRN_PV_LAG", "2"))  # groups PV trails behind QK

_CACHE = {}


def _build_module():
    """Build + compile the Bass module (once per process per variant)."""
    key = ("nc", DT, GROUP, LG_BUFS, TRIM, WARMUP, PT_BUFS, DTRIM, ALT, PAD_GROUPS, PAD_N, PAD_W, OT16, PV_BUFS, TAILPOOL, PV_LAG)
    if key in _CACHE:
        return _CACHE[key]

    import concourse.mybir as mybir
    import concourse.tile as tile
    from concourse import bacc

    f32 = mybir.dt.float32
    mmdt = mybir.dt.float16 if DT == "f16" else mybir.dt.float32r
    naug = 2 if DT == "f16" else 0

    nc = bacc.Bacc(
        "TRN2", target_bir_lowering=False, debug=False, num_devices=N_CORES
    )

    otdt = mybir.dt.float16 if OT16 else f32
    qT = nc.dram_tensor("qT", [HPC, D + naug, S], mmdt, kind="ExternalInput").ap()
    kT = nc.dram_tensor("kT", [HPC, D + naug, S], mmdt, kind="ExternalInput").ap()
    # v pre-transposed on host to [128, S//128 * D] per head: contiguous DMA
    v = nc.dram_tensor("v", [HPC, 128, (S // 128) * D], mmdt, kind="ExternalInput").ap()
    maskc = nc.dram_tensor("maskc", [128, 896], mmdt, kind="ExternalInput").ap()
    biasc = nc.dram_tensor("biasc", [128, 1], f32, kind="ExternalInput").ap()
    ot = nc.dram_tensor("ot", [HPC, D, S], otdt, kind="ExternalOutput").ap()

    with tile.TileContext(nc) as tc:
        with (
            tc.tile_pool(name="consts", bufs=1) as consts,
            tc.tile_pool(name="qk_sb", bufs=2) as qk_sb,
            tc.tile_pool(name="v_sb", bufs=2) as v_sb,
            tc.tile_pool(name="pt_sb", bufs=PT_BUFS) as pt_sb,
            tc.tile_pool(name="ot_sb", bufs=2) as ot_sb,
            tc.tile_pool(name="lg_ps", bufs=(1 if ALT else LG_BUFS), space="PSUM") as lg_ps,
            tc.tile_pool(name="lgb_ps", bufs=1, space="PSUM") as lgb_ps,
            tc.tile_pool(name="pv_ps", bufs=(2 if ALT or not TAILPOOL else PV_BUFS), space="PSUM") as pv_ps,
        ):
            # warmup weights via memset (no DMA dependency): PE warm-up can
            # start the moment the sequencers come up, before input DMAs land
            warmsb = consts.tile([128, PAD_W], mmdt, tag="warm")
            nc.vector.memset(warmsb, 0.0)

            masksb = consts.tile([128, 896], mmdt, tag="mask")
            nc.sync.dma_start(out=masksb, in_=maskc)
            biassb = consts.tile([128, 1], f32, tag="bias")
            nc.sync.dma_start(out=biassb, in_=biasc)

            # dense dummy matmuls at start: trip the PE HAM clock-gate to
            # K=8/8 and bridge seamlessly into the first real matmuls.
            # Must be full-128-partition matmuls: partial-array (row_grp)
            # work does not register as PE-busy for the clock gate.
            def pad_mm():
                if TAILPOOL:
                    # dummy MMs live in the 1-bank tail pool so they never
                    # collide with a live po accumulation (pv_ps bufs=1)
                    tl = lgb_ps.tile([128, 1, SPAN], f32, tag="tail")
                    nc.tensor.matmul(
                        tl[:, 0, 0:PAD_W],
                        warmsb[:, 0:128],
                        warmsb[:, 0:PAD_W],
                        start=True,
                        stop=True,
                    )
                else:
                    wps = pv_ps.tile([D, SPAN], f32, tag="po")
                    nc.tensor.matmul(
                        wps[:, 0:PAD_W],
                        warmsb[:, 0:D],
                        warmsb[:, 0:PAD_W],
                        start=True,
                        stop=True,
                    )

            for w in range(WARMUP):
                pad_mm()

            def emit_pv(pend):
                # PV matmuls for a completed exp group (trails the logits of
                # the next group so the in-order PE stream never stalls on ACT)
                po_, pt_, gkk_, nkk_, s_, vsb_, h_ = pend
                ndiag = SPAN // KTILE
                for j, kk in enumerate(gkk_):
                    jd = kk - s_ * ndiag
                    first, last = kk == 0, kk == nkk_ - 1
                    if TRIM and jd >= 0:
                        # corner block [128jd, 128jd+128) masked; cols beyond
                        # fully valid; cols before fully masked -> skipped
                        c1 = (jd + 1) * KTILE
                        if c1 < SPAN:
                            # start=True pends-zero the whole 2KB bank, so
                            # only the first emitted piece may carry it
                            nc.tensor.matmul(
                                po_[:, c1:SPAN],
                                vsb_[:, kk, :],
                                pt_[:, j, c1:SPAN],
                                start=first,
                                stop=False,
                            )
                        nc.tensor.matmul(
                            po_[:, jd * KTILE : c1],
                            vsb_[:, kk, :],
                            pt_[:, j, jd * KTILE : c1],
                            start=first and c1 >= SPAN,
                            stop=last,
                        )
                    else:
                        c0 = jd * KTILE if (DTRIM and jd > 0) else 0
                        nc.tensor.matmul(
                            po_[:, c0:SPAN],
                            vsb_[:, kk, :],
                            pt_[:, j, c0:SPAN],
                            start=first,
                            stop=last,
                        )
                if gkk_[-1] == nkk_ - 1:  # span finished
                    oT = ot_sb.tile([D, SPAN], mmdt if OT16 else f32, tag="oT")
                    nc.vector.tensor_copy(oT, po_)
                    nc.sync.dma_start(
                        out=ot[h_, :, s_ * SPAN : (s_ + 1) * SPAN], in_=oT
                    )

            pending = []  # PV trails QK by up to PV_LAG groups (ACT slack)
            alt_par = [0]
            gcount = [0]
            for h in range(HPC):
                qta = qk_sb.tile([D + naug, S], mmdt, tag="qta")
                kta = qk_sb.tile([D + naug, S], mmdt, tag="kta")
                nc.sync.dma_start(out=qta, in_=qT[h])
                nc.sync.dma_start(out=kta, in_=kT[h])

                vsb = v_sb.tile([128, S // 128, D], mmdt, tag="vsb")
                nc.sync.dma_start(
                    out=vsb, in_=v[h].rearrange("p (t d) -> p t d", d=D)
                )

                for s in range(NSPAN):
                    ndiag = SPAN // KTILE
                    nkk = (s + 1) * ndiag  # causal: key tiles 0..nkk-1
                    po = pv_ps.tile([D, SPAN], f32, tag="po")
                    qspan = qta[:, s * SPAN : (s + 1) * SPAN]
                    # chunk the kk list; with ALT, alternate between a 4-bank
                    # and a 2-bank PSUM tile so ACTIVATE count drops while
                    # PE/ACT still double-buffer across the two pools
                    chunks = []
                    g0 = 0
                    while g0 < nkk:
                        if ALT:
                            size = 4 if alt_par[0] == 0 else 2
                            alt_par[0] ^= 1
                        else:
                            size = GROUP
                        chunks.append(list(range(g0, min(g0 + size, nkk))))
                        g0 += size
                    for gkk in chunks:
                        n = len(gkk)
                        if ALT:
                            if len(gkk) > 2:
                                pl = lg_ps.tile([128, 4, SPAN], f32, tag="pl")
                            else:
                                pl = lgb_ps.tile([128, 2, SPAN], f32, tag="plb")
                        elif TAILPOOL and len(gkk) == 1:
                            # size-1 span-tail group: own bank, keeps the main
                            # 2-buf pl rotation free for the next span's QK
                            pl = lgb_ps.tile([128, 1, SPAN], f32, tag="tail")
                        else:
                            pl = lg_ps.tile([128, GROUP, SPAN], f32, tag="pl")
                        gjd0 = gkk[0] - s * ndiag
                        gc0 = gjd0 * KTILE if (DTRIM and gjd0 > 0) else 0
                        for j, kk in enumerate(gkk):
                            # trim only to the group-common dead prefix so the
                            # grouped exp below reads fully-written PSUM
                            nc.tensor.matmul(
                                pl[:, j, gc0:SPAN],
                                kta[:, kk * KTILE : (kk + 1) * KTILE],
                                qspan[:, gc0:SPAN],
                                start=True,
                                stop=True,
                            )
                        while len(pending) >= PV_LAG:
                            emit_pv(pending.pop(0))
                        if h == 0 and gcount[0] < PAD_GROUPS:
                            # keep the PE busy-window saturated through the
                            # pipeline-fill phase so the HAM clock-gate never
                            # sees an idle window and re-throttles
                            gcount[0] += 1
                            for _ in range(PAD_N):
                                pad_mm()
                        pt = pt_sb.tile(
                            [128, max(GROUP, 4 if ALT else 0), SPAN], mmdt, tag="pt"
                        )
                        jd0 = gkk[0] - s * ndiag
                        ec0 = jd0 * KTILE if (DTRIM and jd0 > 0) else 0
                        nc.scalar.activation(
                            pt[:, 0:n, ec0:SPAN],
                            pl[:, 0:n, ec0:SPAN],
                            mybir.ActivationFunctionType.Exp,
                            bias=biassb,
                            scale=EXP_SCALE,
                        )
                        for j, kk in enumerate(gkk):
                            jd = kk - s * ndiag
                            if jd >= 0:  # diagonal tile -> causal mask
                                if TRIM:
                                    # mask only the triangular corner block
                                    nc.vector.tensor_mul(
                                        pt[:, j, jd * KTILE : (jd + 1) * KTILE],
                                        pt[:, j, jd * KTILE : (jd + 1) * KTILE],
                                        masksb[:, 384:512],
                                    )
                                elif DTRIM:
                                    mc0 = jd * KTILE
                                    nc.vector.tensor_mul(
                                        pt[:, j, mc0:SPAN],
                                        pt[:, j, mc0:SPAN],
                                        masksb[:, 384 : 896 - mc0],
                                    )
                                else:
                                    c0 = 384 - 128 * jd
                                    nc.vector.tensor_mul(
                                        pt[:, j, :],
                                        pt[:, j, :],
                                        masksb[:, c0 : c0 + SPAN],
                                    )
                        pending.append((po, pt, gkk, nkk, s, vsb, h))
            for p in pending:
                emit_pv(p)

    nc.compile()
    _CACHE[key] = nc
    return nc


def _host_prep(q, k, v):
    """Shard + relayout inputs for the 8 cores. Returns (in_maps, row_scale)."""
    q = np.ascontiguousarray(np.asarray(q, dtype=np.float32)).reshape(B * H, S, D)
    k = np.ascontiguousarray(np.asarray(k, dtype=np.float32)).reshape(B * H, S, D)
    v = np.ascontiguousarray(np.asarray(v, dtype=np.float32)).reshape(B * H, S, D)

    qsq = (q.astype(np.float32) ** 2).sum(-1)  # [BH, S]
    ksq = (k.astype(np.float32) ** 2).sum(-1)

    if DT == "f16":
        npdt = np.float16
        # pt <= e^{0.125*max(qsq) + C}; keep under ~e^{10.5} (fp16 max 65504)
        C = float(min(10.5 - SM * qsq.max(), 0.0))
        qT = np.zeros((B * H, D + 2, S), np.float16)
        kT = np.zeros((B * H, D + 2, S), np.float16)
        qT[:, :D, :] = q.transpose(0, 2, 1)
        kT[:, :D, :] = k.transpose(0, 2, 1)
        qT[:, D, :] = -0.5
        qT[:, D + 1, :] = -0.5
        khi = ksq.astype(np.float16)
        klo = (ksq - khi.astype(np.float32)).astype(np.float16)
        kT[:, D, :] = khi
        kT[:, D + 1, :] = klo
        vin = v.astype(np.float16)
        # device layout [BH, 128, (S//128)*D]: partition-contiguous V tiles
        vin = np.ascontiguousarray(
            vin.reshape(B * H, S // 128, 128, D).transpose(0, 2, 1, 3)
        ).reshape(B * H, 128, (S // 128) * D)
        # host applies D_q * e^{-C}
        row_scale = np.exp(-SM * qsq.astype(np.float64) - C).astype(np.float32)
    else:
        npdt = np.float32
        C = 0.0
        qT = np.ascontiguousarray(q.transpose(0, 2, 1))
        kT = np.ascontiguousarray(k.transpose(0, 2, 1))
        dk = np.exp(-SM * ksq.astype(np.float64)).astype(np.float32)
        vin = v * dk[:, :, None]  # V' = D_k V
        vin = np.ascontiguousarray(
            vin.reshape(B * H, S // 128, 128, D).transpose(0, 2, 1, 3)
        ).reshape(B * H, 128, (S // 128) * D)
        row_scale = np.exp(-SM * qsq.astype(np.float64)).astype(np.float32)

    # maskc[r, c] = 1 if c >= r + 384 else 0 ; slice [384-128j : 896-128j]
    # gives the causal mask for diagonal tile offset j; [384:512] is the
    # corner-block mask (q_local >= k_local)
    r = np.arange(128)[:, None]
    c = np.arange(896)[None, :]
    maskc = (c >= r + 384).astype(npdt)
    biasc = np.full((128, 1), C, dtype=np.float32)

    in_maps = []
    for core in range(N_CORES):
        sl = slice(core * HPC, (core + 1) * HPC)
        in_maps.append(
            {
                "qT": np.ascontiguousarray(qT[sl]),
                "kT": np.ascontiguousarray(kT[sl]),
                "v": np.ascontiguousarray(vin[sl]),
                "maskc": maskc,
                "biasc": biasc,
            }
        )
    return in_maps, row_scale


def _gather(results, row_scale):
    """results[core]["ot"] : [HPC, D, S] -> full [B, H, S, D] (applies D_q)."""
    outs = [np.asarray(r["ot"]) for r in results]
    o = np.concatenate(outs, axis=0)  # [BH, D, S]
    o = o.transpose(0, 2, 1) * row_scale[:, :, None]  # [BH, S, D]
    return np.ascontiguousarray(o.reshape(B, H, S, D).astype(np.float32))


def kernel(q, k, v):
    from concourse.bass_utils import run_bass_kernel_spmd

    nc = _build_module()
    in_maps, row_scale = _host_prep(q, k, v)
    res = run_bass_kernel_spmd(nc, in_maps, core_ids=list(range(N_CORES)))
    return _gather(res.results, row_scale)


if __name__ == "__main__":
    rng = np.random.default_rng(0)
    q = rng.standard_normal((B, H, S, D), dtype=np.float32)
    k = rng.standard_normal((B, H, S, D), dtype=np.float32)
    v = rng.standard_normal((B, H, S, D), dtype=np.float32)
    o = kernel(q, k, v)
    print("out", o.shape, o.dtype, float(np.abs(o).max()))



# revision 25
# speedup vs baseline: 1.2777x; 1.0084x over previous
"""Trainium2 Bass kernel for causal RBF (squared-exponential) attention.

  p_ij = exp(-sm * ||q_i - k_j||^2) causal-masked, out = p @ v (no normalization)
  B,H,S,D = 2,16,2048,64 ; sm = 0.125

Sharding: B*H = 32 heads, 4 heads per core across 8 NeuronCores (head
parallel, no cross-core comm).

Factorization: p = D_q . exp(2*sm*qk - sm*||k||^2) with D_q diagonal in
exp(-sm*||q||^2). D_q is applied to the output rows on the host (O(N));
the device computes, per 128-key x 512-query tile, one fp16 matmul with a
66-deep contraction (rows 0..63 = K^T/Q^T, rows 64/65 = ksq split hi/lo
against -1/2 const rows), then ScalarE evaluates
    pt = exp(0.25 * psum + C)       (psum = qk - ksq/2)
straight PSUM -> SBUF. C is a host-supplied bias chosen from max(qsq) so
that pt <= e^{0.125 qsq + C} stays inside fp16 range (p <= 1 identity).
Diagonal tiles: fully-masked columns are skipped in PV entirely; only the
128x128 triangular corner is masked (VectorE) before the PV matmul.
PV: out^T[64, 512] += V^T_kk @ P^T_kk accumulated in PSUM over kk.
out^T goes back in [D, S] layout (fp16 by default: po <= ~1.4e3, well
inside fp16, halves output DMA); the host transposes and applies
D_q * e^{-C}. All O(S^2) work stays on device.

I/O hygiene (rev 2026-08-08): V is pre-transposed on host to
[128, (S/128)*D] per head so the device DMA is partition-contiguous
(was a (t p) d -> p t d DRAM gather = ~10k 128B descriptors clogging
all 16 queues, delaying output drain by ~12us). Warmup weights come
from a VectorE memset instead of the mask DMA so PE warmup issues
~2.5us earlier. Keep PAD_W=512: thinner pads (256) weaken the HAM
bridge and cost 10-30us. Chip power-throttling (P0) ~60us into a run
adds 10-25% run-to-run variance; compare medians across runs.

Scheduling (135.8 -> ~109us): (1) TAILPOOL: size-1 span-tail logits
groups + all warmup/pad dummy MMs go to a dedicated 1-bank PSUM pool
(paid for by single-buffering po) so the main 2-buf pl rotation never
blocks the next span's QK — killed the 700-860ns boundary stalls and
collapsed run-to-run variance. (2) PV_LAG=2: PV matmuls trail the QK
stream by two groups (pending deque), giving ScalarE's exp enough
slack that PV never waits on pt.
"""

import os
import sys

if "/opt/trn_rl_repo" not in sys.path:
    sys.path.insert(0, "/opt/trn_rl_repo")

import numpy as np

B, H, S, D = 2, 16, 2048, 64
SM = 0.125
N_CORES = 8
HPC = (B * H) // N_CORES  # heads per core = 4
SPAN = 512  # query-span per PSUM accumulation group
NSPAN = S // SPAN  # 4
KTILE = 128  # key rows per logits tile
EXP_SCALE = 2.0 * SM  # 0.25

# knobs
DT = os.environ.get("KRN_DT", "f16")  # "f16" | "f32r"
GROUP = int(os.environ.get("KRN_GROUP", "3"))  # logits tiles per ACTIVATE
LG_BUFS = int(os.environ.get("KRN_LG_BUFS", "2"))  # logits psum buffers
TRIM = os.environ.get("KRN_TRIM", "0") == "1"  # diag-tile column trim
WARMUP = int(os.environ.get("KRN_WARMUP", "20"))  # PE warmup matmuls
DTRIM = os.environ.get("KRN_DTRIM", "1") == "1"  # skip dead col-prefix of diag tiles
ALT = os.environ.get("KRN_ALT", "0") == "1"  # alternating 4+2 logits psum pools
PAD_GROUPS = int(os.environ.get("KRN_PAD_GROUPS", "8"))  # early groups padded
PAD_N = int(os.environ.get("KRN_PAD_N", "3"))  # dummy matmuls per padded group
PAD_W = int(os.environ.get("KRN_PAD_W", "512"))  # pad/warmup matmul stream width
PT_BUFS = int(os.environ.get("KRN_PT_BUFS", "6"))
OT16 = os.environ.get("KRN_OT16", "1") == "1"  # fp16 output DMA
PV_BUFS = int(os.environ.get("KRN_PV_BUFS", "1"))  # po psum buffers
TAILPOOL = os.environ.get("KRN_TAILPOOL", "1") == "1"  # 1-bank pool for size-1 groups+pads
PV_LAG = int(os.environ.get("KRN_PV_LAG", "2"))  # groups PV trails behind QK

_CACHE = {}


def _build_module():
    """Build + compile the Bass module (once per process per variant)."""
    key = ("nc", DT, GROUP, LG_BUFS, TRIM, WARMUP, PT_BUFS, DTRIM, ALT, PAD_GROUPS, PAD_N, PAD_W, OT16, PV_BUFS, TAILPOOL, PV_LAG)
    if key in _CACHE:
        return _CACHE[key]

    import concourse.mybir as mybir
    import concourse.tile as tile
    from concourse import bacc

    f32 = mybir.dt.float32
    mmdt = mybir.dt.float16 if DT == "f16" else mybir.dt.float32r
    naug = 2 if DT == "f16" else 0

    nc = bacc.Bacc(
        "TRN2", target_bir_lowering=False, debug=False, num_devices=N_CORES
    )

    otdt = mybir.dt.float16 if OT16 else f32
    qT = nc.dram_tensor("qT", [HPC, D + naug, S], mmdt, kind="ExternalInput").ap()
    kT = nc.dram_tensor("kT", [HPC, D + naug, S], mmdt, kind="ExternalInput").ap()
    # v pre-transposed on host to [128, S//128 * D] per head: contiguous DMA
    v = nc.dram_tensor("v", [HPC, 128, (S // 128) * D], mmdt, kind="ExternalInput").ap()
    maskc = nc.dram_tensor("maskc", [128, 896], mmdt, kind="ExternalInput").ap()
    biasc = nc.dram_tensor("biasc", [128, 1], f32, kind="ExternalInput").ap()
    ot = nc.dram_tensor("ot", [HPC, D, S], otdt, kind="ExternalOutput").ap()

    with tile.TileContext(nc) as tc:
        with (
            tc.tile_pool(name="consts", bufs=1) as consts,
            tc.tile_pool(name="qk_sb", bufs=2) as qk_sb,
            tc.tile_pool(name="v_sb", bufs=2) as v_sb,
            tc.tile_pool(name="pt_sb", bufs=PT_BUFS) as pt_sb,
            tc.tile_pool(name="ot_sb", bufs=2) as ot_sb,
            tc.tile_pool(name="lg_ps", bufs=(1 if ALT else LG_BUFS), space="PSUM") as lg_ps,
            tc.tile_pool(name="lgb_ps", bufs=1, space="PSUM") as lgb_ps,
            tc.tile_pool(name="pv_ps", bufs=(2 if ALT or not TAILPOOL else PV_BUFS), space="PSUM") as pv_ps,
        ):
            # warmup weights via memset (no DMA dependency): PE warm-up can
            # start the moment the sequencers come up, before input DMAs land
            warmsb = consts.tile([128, PAD_W], mmdt, tag="warm")
            nc.vector.memset(warmsb, 0.0)

            masksb = consts.tile([128, 896], mmdt, tag="mask")
            nc.sync.dma_start(out=masksb, in_=maskc)
            biassb = consts.tile([128, 1], f32, tag="bias")
            nc.sync.dma_start(out=biassb, in_=biasc)

            # dense dummy matmuls at start: trip the PE HAM clock-gate to
            # K=8/8 and bridge seamlessly into the first real matmuls.
            # Must be full-128-partition matmuls: partial-array (row_grp)
            # work does not register as PE-busy for the clock gate.
            def pad_mm():
                if TAILPOOL:
                    # dummy MMs live in the 1-bank tail pool so they never
                    # collide with a live po accumulation (pv_ps bufs=1)
                    tl = lgb_ps.tile([128, 1, SPAN], f32, tag="tail")
                    nc.tensor.matmul(
                        tl[:, 0, 0:PAD_W],
                        warmsb[:, 0:128],
                        warmsb[:, 0:PAD_W],
                        start=True,
                        stop=True,
                    )
                else:
                    wps = pv_ps.tile([D, SPAN], f32, tag="po")
                    nc.tensor.matmul(
                        wps[:, 0:PAD_W],
                        warmsb[:, 0:D],
                        warmsb[:, 0:PAD_W],
                        start=True,
                        stop=True,
                    )

            for w in range(WARMUP):
                pad_mm()

            def emit_pv(pend):
                # PV matmuls for a completed exp group (trails the logits of
                # the next group so the in-order PE stream never stalls on ACT)
                po_, pt_, gkk_, nkk_, s_, vsb_, h_ = pend
                ndiag = SPAN // KTILE
                for j, kk in enumerate(gkk_):
                    jd = kk - s_ * ndiag
                    first, last = kk == 0, kk == nkk_ - 1
                    if TRIM and jd >= 0:
                        # corner block [128jd, 128jd+128) masked; cols beyond
                        # fully valid; cols before fully masked -> skipped
                        c1 = (jd + 1) * KTILE
                        if c1 < SPAN:
                            # start=True pends-zero the whole 2KB bank, so
                            # only the first emitted piece may carry it
                            nc.tensor.matmul(
                                po_[:, c1:SPAN],
                                vsb_[:, kk, :],
                                pt_[:, j, c1:SPAN],
                                start=first,
                                stop=False,
                            )
                        nc.tensor.matmul(
                            po_[:, jd * KTILE : c1],
                            vsb_[:, kk, :],
                            pt_[:, j, jd * KTILE : c1],
                            start=first and c1 >= SPAN,
                            stop=last,
                        )
                    else:
                        c0 = jd * KTILE if (DTRIM and jd > 0) else 0
                        nc.tensor.matmul(
                            po_[:, c0:SPAN],
                            vsb_[:, kk, :],
                            pt_[:, j, c0:SPAN],
                            start=first,
                            stop=last,
                        )
                if gkk_[-1] == nkk_ - 1:  # span finished
                    oT = ot_sb.tile([D, SPAN], mmdt if OT16 else f32, tag="oT")
                    nc.vector.tensor_copy(oT, po_)
                    nc.sync.dma_start(
                        out=ot[h_, :, s_ * SPAN : (s_ + 1) * SPAN], in_=oT
                    )

            pending = []  # PV trails QK by up to PV_LAG groups (ACT slack)
            alt_par = [0]
            gcount = [0]
            for h in range(HPC):
                qta = qk_sb.tile([D + naug, S], mmdt, tag="qta")
                kta = qk_sb.tile([D + naug, S], mmdt, tag="kta")
                nc.sync.dma_start(out=qta, in_=qT[h])
                nc.sync.dma_start(out=kta, in_=kT[h])

                vsb = v_sb.tile([128, S // 128, D], mmdt, tag="vsb")
                nc.sync.dma_start(
                    out=vsb, in_=v[h].rearrange("p (t d) -> p t d", d=D)
                )

                for s in range(NSPAN):
                    ndiag = SPAN // KTILE
                    nkk = (s + 1) * ndiag  # causal: key tiles 0..nkk-1
                    po = pv_ps.tile([D, SPAN], f32, tag="po")
                    qspan = qta[:, s * SPAN : (s + 1) * SPAN]
                    # chunk the kk list; with ALT, alternate between a 4-bank
                    # and a 2-bank PSUM tile so ACTIVATE count drops while
                    # PE/ACT still double-buffer across the two pools
                    chunks = []
                    g0 = 0
                    while g0 < nkk:
                        if ALT:
                            size = 4 if alt_par[0] == 0 else 2
                            alt_par[0] ^= 1
                        else:
                            size = GROUP
                        chunks.append(list(range(g0, min(g0 + size, nkk))))
                        g0 += size
                    for gkk in chunks:
                        n = len(gkk)
                        if ALT:
                            if len(gkk) > 2:
                                pl = lg_ps.tile([128, 4, SPAN], f32, tag="pl")
                            else:
                                pl = lgb_ps.tile([128, 2, SPAN], f32, tag="plb")
                        elif TAILPOOL and len(gkk) == 1:
                            # size-1 span-tail group: own bank, keeps the main
                            # 2-buf pl rotation free for the next span's QK
                            pl = lgb_ps.tile([128, 1, SPAN], f32, tag="tail")
                        else:
                            pl = lg_ps.tile([128, GROUP, SPAN], f32, tag="pl")
                        gjd0 = gkk[0] - s * ndiag
                        gc0 = gjd0 * KTILE if (DTRIM and gjd0 > 0) else 0
                        for j, kk in enumerate(gkk):
                            # trim only to the group-common dead prefix so the
                            # grouped exp below reads fully-written PSUM
                            nc.tensor.matmul(
                                pl[:, j, gc0:SPAN],
                                kta[:, kk * KTILE : (kk + 1) * KTILE],
                                qspan[:, gc0:SPAN],
                                start=True,
                                stop=True,
                            )
                        while len(pending) >= PV_LAG:
                            emit_pv(pending.pop(0))
                        if h == 0 and gcount[0] < PAD_GROUPS:
                            # keep the PE busy-window saturated through the
                            # pipeline-fill phase so the HAM clock-gate never
                            # sees an idle window and re-throttles
                            gcount[0] += 1
                            for _ in range(PAD_N):
                                pad_mm()
                        pt = pt_sb.tile(
                            [128, max(GROUP, 4 if ALT else 0), SPAN], mmdt, tag="pt"
                        )
                        jd0 = gkk[0] - s * ndiag
                        ec0 = jd0 * KTILE if (DTRIM and jd0 > 0) else 0
                        nc.scalar.activation(
                            pt[:, 0:n, ec0:SPAN],
                            pl[:, 0:n, ec0:SPAN],
                            mybir.ActivationFunctionType.Exp,
                            bias=biassb,
                            scale=EXP_SCALE,
                        )
                        for j, kk in enumerate(gkk):
                            jd = kk - s * ndiag
                            if jd >= 0:  # diagonal tile -> causal mask
                                if TRIM:
                                    # mask only the triangular corner block
                                    nc.vector.tensor_mul(
                                        pt[:, j, jd * KTILE : (jd + 1) * KTILE],
                                        pt[:, j, jd * KTILE : (jd + 1) * KTILE],
                                        masksb[:, 384:512],
                                    )
                                elif DTRIM:
                                    mc0 = jd * KTILE
                                    nc.vector.tensor_mul(
                                        pt[:, j, mc0:SPAN],
                                        pt[:, j, mc0:SPAN],
                                        masksb[:, 384 : 896 - mc0],
                                    )
                                else:
                                    c0 = 384 - 128 * jd
                                    nc.vector.tensor_mul(
                                        pt[:, j, :],
                                        pt[:, j, :],
                                        masksb[:, c0 : c0 + SPAN],
                                    )
                        pending.append((po, pt, gkk, nkk, s, vsb, h))
            for p in pending:
                emit_pv(p)

    nc.compile()
    _CACHE[key] = nc
    return nc


def _host_prep(q, k, v):
    """Shard + relayout inputs for the 8 cores. Returns (in_maps, row_scale)."""
    q = np.ascontiguousarray(np.asarray(q, dtype=np.float32)).reshape(B * H, S, D)
    k = np.ascontiguousarray(np.asarray(k, dtype=np.float32)).reshape(B * H, S, D)
    v = np.ascontiguousarray(np.asarray(v, dtype=np.float32)).reshape(B * H, S, D)

    qsq = (q.astype(np.float32) ** 2).sum(-1)  # [BH, S]
    ksq = (k.astype(np.float32) ** 2).sum(-1)

    if DT == "f16":
        npdt = np.float16
        # pt <= e^{0.125*max(qsq) + C}; keep under ~e^{10.5} (fp16 max 65504)
        C = float(min(10.5 - SM * qsq.max(), 0.0))
        qT = np.zeros((B * H, D + 2, S), np.float16)
        kT = np.zeros((B * H, D + 2, S), np.float16)
        qT[:, :D, :] = q.transpose(0, 2, 1)
        kT[:, :D, :] = k.transpose(0, 2, 1)
        qT[:, D, :] = -0.5
        qT[:, D + 1, :] = -0.5
        khi = ksq.astype(np.float16)
        klo = (ksq - khi.astype(np.float32)).astype(np.float16)
        kT[:, D, :] = khi
        kT[:, D + 1, :] = klo
        vin = v.astype(np.float16)
        # device layout [BH, 128, (S//128)*D]: partition-contiguous V tiles
        vin = np.ascontiguousarray(
            vin.reshape(B * H, S // 128, 128, D).transpose(0, 2, 1, 3)
        ).reshape(B * H, 128, (S // 128) * D)
        # host applies D_q * e^{-C}
        row_scale = np.exp(-SM * qsq.astype(np.float64) - C).astype(np.float32)
    else:
        npdt = np.float32
        C = 0.0
        qT = np.ascontiguousarray(q.transpose(0, 2, 1))
        kT = np.ascontiguousarray(k.transpose(0, 2, 1))
        dk = np.exp(-SM * ksq.astype(np.float64)).astype(np.float32)
        vin = v * dk[:, :, None]  # V' = D_k V
        vin = np.ascontiguousarray(
            vin.reshape(B * H, S // 128, 128, D).transpose(0, 2, 1, 3)
        ).reshape(B * H, 128, (S // 128) * D)
        row_scale = np.exp(-SM * qsq.astype(np.float64)).astype(np.float32)

    # maskc[r, c] = 1 if c >= r + 384 else 0 ; slice [384-128j : 896-128j]
    # gives the causal mask for diagonal tile offset j; [384:512] is the
    # corner-block mask (q_local >= k_local)
    r = np.arange(128)[:, None]
    c = np.arange(896)[None, :]
    maskc = (c >= r + 384).astype(npdt)
    biasc = np.full((128, 1), C, dtype=np.float32)

    in_maps = []
    for core in range(N_CORES):
        sl = slice(core * HPC, (core + 1) * HPC)
        in_maps.append(
            {
                "qT": np.ascontiguousarray(qT[sl]),
                "kT": np.ascontiguousarray(kT[sl]),
                "v": np.ascontiguousarray(vin[sl]),
                "maskc": maskc,
                "biasc": biasc,
            }
        )
    return in_maps, row_scale


def _gather(results, row_scale):
    """results[core]["ot"] : [HPC, D, S] -> full [B, H, S, D] (applies D_q)."""
    outs = [np.asarray(r["ot"]) for r in results]
    o = np.concatenate(outs, axis=0)  # [BH, D, S]
    o = o.transpose(0, 2, 1) * row_scale[:, :, None]  # [BH, S, D]
    return np.ascontiguousarray(o.reshape(B, H, S, D).astype(np.float32))


def kernel(q, k, v):
    from concourse.bass_utils import run_bass_kernel_spmd

    nc = _build_module()
    in_maps, row_scale = _host_prep(q, k, v)
    res = run_bass_kernel_spmd(nc, in_maps, core_ids=list(range(N_CORES)))
    return _gather(res.results, row_scale)


if __name__ == "__main__":
    rng = np.random.default_rng(0)
    q = rng.standard_normal((B, H, S, D), dtype=np.float32)
    k = rng.standard_normal((B, H, S, D), dtype=np.float32)
    v = rng.standard_normal((B, H, S, D), dtype=np.float32)
    o = kernel(q, k, v)
    print("out", o.shape, o.dtype, float(np.abs(o).max()))

